# revision 1
# baseline (speedup 1.0000x reference)
"""Multi-head self-attention with RoPE on 8 Trainium2 NeuronCores.

Full inputs in, full output out. Sharding: batch (2) x head-groups (4 heads
per core). Each core computes qkv projections for its heads, RoPE, full
softmax(QK^T)V, and a partial output projection; host sums the 4 partials
per batch element and adds b_out.

Problem shape: B=2, T=2048, D=1024, H=16, HD=64 (hardcoded).
"""

import numpy as np
from contextlib import ExitStack

import concourse.bass as bass
import concourse.mybir as mybir
import concourse.tile as tile
from concourse import bass_utils

B, T, D, H = 2, 2048, 1024, 16
HD = 64          # head dim
HL = 4           # heads per core
N_CORES = 8
ROPE_BASE = 10000.0

F32 = mybir.dt.float32
F32R = mybir.dt.float32r
BF16 = mybir.dt.bfloat16

Exp = mybir.ActivationFunctionType.Exp

# results of the last run (for test harness introspection)
LAST_RESULTS = None
TRACE = False


def _split_excess_waits(nc, cap=1):
    """walrus in this env rejects >1 sync-wait per instruction; split extras
    onto single-wait NoOps on the same engine queue."""
    n = 0
    for f in nc.m.functions:
        for bb in f.blocks:
            insts = bb.instructions
            if not any(
                i.sync_info is not None and len(i.sync_info.on_wait) > cap
                for i in insts
            ):
                continue
            out = []
            for inst in insts:
                si = inst.sync_info
                waits = list(si.on_wait) if si is not None else []
                if len(waits) > cap:
                    extra, keep = waits[:-cap], waits[-cap:]
                    for k, w in enumerate(extra):
                        nop = mybir.InstNoOp(
                            name=f"{inst.name}-ws{k}",
                            engine=inst.engine,
                            sync_info=mybir.SyncInfo(on_wait=[w], on_update=[]),
                            bass_nofuse=True,
                        )
                        nc.register_instruction(nop)
                        out.append(nop)
                        n += 1
                    inst.sync_info = mybir.SyncInfo(
                        on_wait=keep, on_update=list(si.on_update)
                    )
                out.append(inst)
            bb.instructions = out
    return n


def _build_bass(with_qkv_bias, with_v_bias):
    nc = bass.Bass("TRN2", target_bir_lowering=False, debug=False, num_devices=1)

    # ---- DRAM I/O ----
    d_xT = nc.dram_tensor("xT", [D, T], F32R, kind="ExternalInput").ap()
    d_wqk = nc.dram_tensor("wqk", [D, 2 * HL * HD], F32R, kind="ExternalInput").ap()
    d_wv = nc.dram_tensor("wv", [D, HL * (HD + 1)], F32R, kind="ExternalInput").ap()
    d_bqk = nc.dram_tensor("bqk", [1, 2 * HL * HD], F32R, kind="ExternalInput").ap()
    d_bv = nc.dram_tensor("bv", [1, HL * (HD + 1)], F32R, kind="ExternalInput").ap()
    d_ones = nc.dram_tensor("ones", [1, T], F32R, kind="ExternalInput").ap()
    d_cos = nc.dram_tensor("cos2", [128, T], F32, kind="ExternalInput").ap()
    d_sin = nc.dram_tensor("sin2", [128, T], F32, kind="ExternalInput").ap()
    d_rT = nc.dram_tensor("rT", [128, 128], F32R, kind="ExternalInput").ap()
    d_ind = nc.dram_tensor("ind", [2, 128], F32R, kind="ExternalInput").ap()
    d_amask = nc.dram_tensor("amask", [128, T // 128], F32, kind="ExternalInput").ap()
    d_wo = nc.dram_tensor("wo", [HL * HD, D], F32R, kind="ExternalInput").ap()
    d_zeros = nc.dram_tensor("zeros", [HD, T], F32R, kind="ExternalInput").ap()
    d_out = nc.dram_tensor("out_part", [2, T, D], F32, kind="ExternalOutput").ap()

    NT = T // 128            # 16 token tiles
    NK = D // 128            # 8 contraction chunks
    SC = HD ** -0.5          # softmax scale

    with tile.TileContext(nc) as tc, ExitStack() as ctx:
        pool = lambda st, name, bufs: st.enter_context(tc.tile_pool(name=name, bufs=bufs))
        psum = lambda st, name, bufs: st.enter_context(
            tc.tile_pool(name=name, bufs=bufs, space="PSUM")
        )

        # lifetime-grouped pools: g_load dies after phase 2, g_att2/g_fin open late
        g_load = ctx.enter_context(ExitStack())
        psA = ctx.enter_context(ExitStack())

        p_const = pool(ctx, "const", 1)
        p_qkT = pool(ctx, "qkT", 2)
        p_kpad = pool(ctx, "kpad", 4)
        p_v = pool(ctx, "v", NT)
        p_xt = pool(g_load, "xt", NK)
        p_w = pool(g_load, "w", NK)
        p_wv = pool(g_load, "wv", NK)
        p_cs = pool(g_load, "cossin", 1)
        p_tmp = pool(g_load, "tmp", 2)

        ps_qk = psum(psA, "ps_qk", 2)
        ps_rot = psum(psA, "ps_rot", 1)
        ps_v = psum(psA, "ps_v", 2)

        # ---- weight / input loads ----
        xt = []
        wqk = []
        wv = []
        for k in range(NK):
            tk = p_xt.tile([128, T], F32R, tag="xt")
            for q4 in range(4):
                s4 = slice(q4 * 512, (q4 + 1) * 512)
                nc.sync.dma_start(tk[:, s4], d_xT[k * 128:(k + 1) * 128, s4])
            xt.append(tk)
            tw = p_w.tile([128, 2 * HL * HD], F32R, tag="wqk")
            nc.sync.dma_start(tw[:], d_wqk[k * 128:(k + 1) * 128, :])
            wqk.append(tw)
            tv = p_wv.tile([128, HL * (HD + 1)], F32R, tag="wv")
            nc.sync.dma_start(tv[:], d_wv[k * 128:(k + 1) * 128, :])
            wv.append(tv)

        # ---- constants / tables ----
        t_ones = p_const.tile([1, 512], F32R, tag="ones")
        nc.sync.dma_start(t_ones[:], d_ones[:, 0:512])
        t_bqk = p_const.tile([1, 2 * HL * HD], F32R, tag="bqk")
        nc.sync.dma_start(t_bqk[:], d_bqk[:])
        t_bv = p_const.tile([1, HL * (HD + 1)], F32R, tag="bv")
        nc.sync.dma_start(t_bv[:], d_bv[:])
        t_cos = p_cs.tile([128, T], F32, tag="cos")
        nc.sync.dma_start(t_cos[:], d_cos[:])
        t_sin = p_cs.tile([128, T], F32, tag="sin")
        nc.sync.dma_start(t_sin[:], d_sin[:])
        t_rT = p_const.tile([128, 128], F32R, tag="rT")
        nc.sync.dma_start(t_rT[:], d_rT[:])
        t_ind2 = p_const.tile([2, 128], F32R, tag="ind")
        nc.sync.dma_start(t_ind2[:], d_ind[:])
        t_amask = p_const.tile([128, T // 128], F32, tag="amask")
        nc.sync.dma_start(t_amask[:], d_amask[:])

        # ---- phase 1: q/k projections (feature-major) + RoPE ----
        # q chunks (c2=0,1) -> qkT[pair]; k chunks (c2=2,3) -> zero-padded
        # per-head tiles kpad[2*pair+{0,1}] so scores can run K=128.
        qkT = []
        kpad = []
        for pair in range(2):
            kA = p_kpad.tile([128, T], F32R, tag="kpad")
            kB = p_kpad.tile([128, T], F32R, tag="kpad")
            nc.sync.dma_start(kA[HD:128, :], d_zeros[:])
            nc.sync.dma_start(kB[0:HD, :], d_zeros[:])
            kpad.append((kA, kB))
        for c2 in range(4):
            is_k = c2 >= 2
            if not is_k:
                t_qk = p_qkT.tile([128, T], F32R, tag="qkT")
                qkT.append(t_qk)
            else:
                kA, kB = kpad[c2 - 2]
            for ih in range(2):  # halves of the token axis
                sl = slice(ih * (T // 2), (ih + 1) * (T // 2))
                pqk = ps_qk.tile([128, T // 2], F32, tag="pqk")
                for k in range(NK):
                    for n5 in range(2):
                        s5 = slice(n5 * 512, (n5 + 1) * 512)
                        g5 = slice(ih * (T // 2) + n5 * 512,
                                   ih * (T // 2) + (n5 + 1) * 512)
                        nc.tensor.matmul(
                            pqk[:, s5],
                            wqk[k][:, c2 * 128:(c2 + 1) * 128],
                            xt[k][:, g5],
                            start=(k == 0),
                            stop=(not with_qkv_bias and k == NK - 1),
                            skip_group_check=True,
                        )
                if with_qkv_bias:
                    for n5 in range(2):
                        s5 = slice(n5 * 512, (n5 + 1) * 512)
                        nc.tensor.matmul(
                            pqk[:, s5],
                            t_bqk[:, c2 * 128:(c2 + 1) * 128],
                            t_ones[:, 0:512],
                            start=False,
                            stop=True,
                            skip_group_check=True,
                        )
                # RoPE: roped = raw*cos + R @ (raw*sin)   (sin is 32-symmetric)
                u_sb = p_tmp.tile([128, T // 2], F32R, tag="u")
                nc.vector.tensor_mul(u_sb[:], pqk[:], t_sin[:, sl])
                prot = ps_rot.tile([128, T // 2], F32, tag="prot")
                for n5 in range(2):
                    s5 = slice(n5 * 512, (n5 + 1) * 512)
                    nc.tensor.matmul(
                        prot[:, s5], t_rT[:], u_sb[:, s5],
                        start=True, stop=True, skip_group_check=True,
                    )
                c_sb = p_tmp.tile([128, T // 2], F32, tag="c")
                nc.vector.tensor_mul(c_sb[:], pqk[:], t_cos[:, sl])
                if not is_k:
                    nc.vector.tensor_add(t_qk[:, sl], c_sb[:], prot[:])
                else:
                    nc.vector.tensor_add(kA[0:HD, sl], c_sb[0:HD, :],
                                         prot[0:HD, :])
                    nc.vector.tensor_add(kB[HD:128, sl], c_sb[HD:128, :],
                                         prot[HD:128, :])

        # ---- phase 2: v projection (token-major, interleaved + ones col) ----
        VW = HL * (HD + 1)  # 260
        v_sb = []
        for t in range(NT):
            pv_ps = ps_v.tile([128, VW], F32, tag="pv_ps")
            for k in range(NK):
                nc.tensor.matmul(
                    pv_ps[:],
                    xt[k][:, t * 128:(t + 1) * 128],
                    wv[k][:],
                    start=(k == 0),
                    stop=(not with_v_bias and k == NK - 1),
                    skip_group_check=True,
                )
            vt = p_v.tile([128, VW], BF16, tag="v")
            if with_v_bias:
                # bias + ones column (bv has 1.0 at the ones slots)
                nc.tensor.matmul(
                    pv_ps[:], t_ones[:, 0:128], t_bv[:],
                    start=False, stop=True, skip_group_check=True,
                )
                nc.vector.tensor_copy(vt[:], pv_ps[:])
            else:
                # wv_aug's ones-slot columns are zero, so pv_ps has zeros
                # there; copy then stamp the ones columns directly.
                nc.vector.tensor_copy(vt[:], pv_ps[:])
                ones_cols = vt[:].rearrange("p (h c) -> p h c", h=HL)[:, :, HD:HD + 1]
                nc.gpsimd.memset(ones_cols, 1.0)
            v_sb.append(vt)

        # ---- phase 3: attention, head pairs row-tiled on the PE array ----
        g_load.close()
        psA.close()
        psC = ctx.enter_context(ExitStack())
        ps_s = psum(psC, "ps_s", 2)
        ps_pv = psum(psC, "ps_pv", 1)
        ps_x = psum(psC, "ps_x", 1)
        p_e = pool(ctx, "eT", 4)
        p_a = pool(ctx, "aT", HL)
        p_fin = ctx.enter_context(ExitStack())
        p_anorm = pool(p_fin, "anorm", 2)
        p_wo = pool(p_fin, "wo", 2)
        p_osb = pool(p_fin, "osb", 2)
        p_small = pool(p_fin, "small", 1)
        wo_sb = []
        for c2 in range(2):
            wt = p_wo.tile([128, D], F32R, tag="wo")
            nc.sync.dma_start(wt[:], d_wo[c2 * 128:(c2 + 1) * 128, :])
            wo_sb.append(wt)
        a_sb = [None] * HL
        anorm = [None, None]
        TH2 = 1024
        for pair in range(2):
            hA, hB = 2 * pair, 2 * pair + 1
            qc = qkT[pair]
            atA = p_a.tile([HD + 1, T], F32, tag="aT")
            atB = p_a.tile([HD + 1, T], F32, tag="aT")
            a_sb[hA], a_sb[hB] = atA, atB
            NTT = T // 128
            sums128 = p_small.tile([128, 2 * NTT], F32, tag=f"sums{pair}")
            for hh in range(2):
                h = 2 * pair + hh
                at = (atA, atB)[hh]
                kp = kpad[pair][hh]
                for ih in range(2):
                    qsl = slice(ih * TH2, (ih + 1) * TH2)
                    pv = ps_pv.tile([HD + 1, TH2], F32, tag="pv")
                    for jb in range(NT):
                        s_ps = ps_s.tile([128, TH2], F32, tag="sT")
                        jsl = slice(jb * 128, (jb + 1) * 128)
                        for n5 in range(2):
                            s5 = slice(n5 * 512, (n5 + 1) * 512)
                            g5 = slice(ih * TH2 + n5 * 512,
                                       ih * TH2 + (n5 + 1) * 512)
                            nc.tensor.matmul(
                                s_ps[:, s5], kp[:, jsl], qc[:, g5],
                                start=True, stop=True, skip_group_check=True,
                            )
                        e_sb = p_e.tile([128, TH2], BF16, tag="eT")
                        nc.scalar.activation(e_sb[:], s_ps[:], Exp,
                                             bias=t_amask[:, jb:jb + 1],
                                             scale=SC)
                        for n5 in range(2):
                            s5 = slice(n5 * 512, (n5 + 1) * 512)
                            nc.tensor.matmul(
                                pv[:, s5],
                                v_sb[jb][:, h * (HD + 1):(h + 1) * (HD + 1)],
                                e_sb[:, s5],
                                start=(jb == 0), stop=(jb == NT - 1),
                                skip_group_check=True,
                            )
                    nc.vector.tensor_copy(at[:, qsl], pv[:])
            for i, at in enumerate((atA, atB)):
                nc.sync.dma_start(
                    sums128[:, i * NTT:(i + 1) * NTT],
                    at[HD:HD + 1, :].rearrange("o (p c) -> o p c", p=128),
                )
            # normalization + projection for this pair; pair 0's work
            # overlaps pair 1's (ACT-bound) attention.
            NTT = T // 128
            recip128 = p_small.tile([128, 2 * NTT], F32, tag=f"recip{pair}")
            nc.vector.reciprocal(recip128[:], sums128[:])
            recip2 = p_small.tile([2, T], F32R, tag=f"recip2_{pair}")
            for i in range(2):
                nc.sync.dma_start(
                    recip2[i:i + 1, :].rearrange("o (p c) -> o p c", p=128),
                    recip128[:, i * NTT:(i + 1) * NTT].bitcast(F32R),
                )
            ar = p_anorm.tile([128, T], F32, tag="anorm_raw")
            nc.sync.dma_start(ar[0:HD, :], atA[0:HD, :])
            nc.sync.dma_start(ar[HD:2 * HD, :], atB[0:HD, :])
            an = p_anorm.tile([128, T], F32R, tag="anorm")
            for ibh in range(2):
                hsl = slice(ibh * (T // 2), (ibh + 1) * (T // 2))
                pb = ps_x.tile([128, T // 2], F32, tag="px")
                for n5 in range(2):
                    s5 = slice(n5 * 512, (n5 + 1) * 512)
                    g5 = slice(ibh * (T // 2) + n5 * 512,
                               ibh * (T // 2) + (n5 + 1) * 512)
                    nc.tensor.matmul(
                        pb[:, s5], t_ind2[:], recip2[:, g5],
                        start=True, stop=True, skip_group_check=True,
                    )
                nc.vector.tensor_mul(an[:, hsl], pb[:], ar[:, hsl])
            anorm[pair] = an

            for t in range(NT):
                if pair == 0:
                    pp = ps_x.tile([128, D], F32, tag="px")
                else:
                    pp = (ps_x if t % 2 == 0 else ps_pv).tile(
                        [128, D], F32, tag=("px" if t % 2 == 0 else "pv"))
                for n5 in range(2):
                    s5 = slice(n5 * 512, (n5 + 1) * 512)
                    nc.tensor.matmul(
                        pp[:, s5],
                        an[:, t * 128:(t + 1) * 128],
                        wo_sb[pair][:, s5],
                        start=True, stop=True, skip_group_check=True,
                    )
                osb = p_osb.tile([128, D], F32, tag="osb")
                if pair == 0:
                    nc.vector.tensor_copy(osb[:], pp[:])
                else:
                    nc.scalar.copy(osb[:], pp[:])
                nc.sync.dma_start(d_out[pair, t * 128:(t + 1) * 128, :], osb[:])

    _split_excess_waits(nc)
    return nc


_NC_CACHE = {}


def _rope_tables():
    inv_freq = (1.0 / (ROPE_BASE ** (np.arange(0, HD, 2, dtype=np.float32) / HD))
                ).astype(np.float32)
    t = np.arange(T, dtype=np.float32)
    freqs = np.einsum("t,f->tf", t, inv_freq).astype(np.float32)  # (T, HD/2)
    emb = np.concatenate([freqs, freqs], axis=-1)                  # (T, HD)
    cosT = np.cos(emb).astype(np.float32).T                        # (HD, T)
    sinT = np.sin(emb).astype(np.float32).T
    cos2 = np.ascontiguousarray(np.tile(cosT, (2, 1)))             # (128, T)
    sin2 = np.ascontiguousarray(np.tile(sinT, (2, 1)))
    return cos2, sin2


def _rot_matrix():
    r = np.zeros((128, 128), dtype=np.float32)
    for p0 in (0, 64):
        for d in range(32):
            r[p0 + d, p0 + 32 + d] = -1.0
            r[p0 + 32 + d, p0 + d] = 1.0
    return np.ascontiguousarray(r.T)


def kernel(x, W_qkv, b_qkv, W_out, b_out, padding_mask):
    global _NC_CACHE, LAST_RESULTS
    x = np.asarray(x, dtype=np.float32)
    W_qkv = np.asarray(W_qkv, dtype=np.float32)
    b_qkv = np.asarray(b_qkv, dtype=np.float32)
    W_out = np.asarray(W_out, dtype=np.float32)
    b_out = np.asarray(b_out, dtype=np.float32)
    padding_mask = np.asarray(padding_mask)

    with_qkv_bias = bool(np.any(b_qkv[:2 * D]))
    with_v_bias = bool(np.any(b_qkv[2 * D:]))
    key = (with_qkv_bias, with_v_bias)
    if key not in _NC_CACHE:
        _NC_CACHE[key] = _build_bass(with_qkv_bias, with_v_bias)
    nc = _NC_CACHE[key]

    cos2, sin2 = _rope_tables()
    rT = _rot_matrix()

    ind = np.zeros((2, 128), dtype=np.float32)
    for f in range(128):
        ind[f // 64, f] = 1.0

    ones = np.ones((1, T), dtype=np.float32)

    in_maps = []
    for c in range(N_CORES):
        b = c // 4
        g = c % 4
        q0 = g * HL * HD
        wq = W_qkv[:, q0:q0 + HL * HD]
        wk = W_qkv[:, D + q0:D + q0 + HL * HD]
        wv_flat = W_qkv[:, 2 * D + q0:2 * D + q0 + HL * HD]
        # interleave v columns with a zero (ones-slot) column per head
        wv_aug = np.zeros((D, HL * (HD + 1)), dtype=np.float32)
        bv_aug = np.zeros((1, HL * (HD + 1)), dtype=np.float32)
        for h in range(HL):
            wv_aug[:, h * (HD + 1):h * (HD + 1) + HD] = wv_flat[:, h * HD:(h + 1) * HD]
            bv_aug[0, h * (HD + 1):h * (HD + 1) + HD] = \
                b_qkv[2 * D + q0 + h * HD:2 * D + q0 + (h + 1) * HD]
            bv_aug[0, h * (HD + 1) + HD] = 1.0
        bqk = np.concatenate(
            [b_qkv[q0:q0 + HL * HD], b_qkv[D + q0:D + q0 + HL * HD]]
        ).reshape(1, -1).astype(np.float32)
        amask = np.where(padding_mask[b], np.float32(-1e30), np.float32(0.0))
        amask = np.ascontiguousarray(amask.reshape(T // 128, 128).T.astype(np.float32))
        in_maps.append({
            "xT": np.ascontiguousarray(x[b].T),
            "wqk": np.ascontiguousarray(np.concatenate([wq, wk], axis=1)),
            "wv": wv_aug,
            "bqk": bqk,
            "bv": bv_aug,
            "ones": ones,
            "cos2": cos2,
            "sin2": sin2,
            "rT": rT,
            "ind": ind,
            "amask": amask,
            "wo": np.ascontiguousarray(W_out[q0:q0 + HL * HD, :]),
            "zeros": np.zeros((HD, T), dtype=np.float32),
        })

    res = bass_utils.run_bass_kernel_spmd(
        nc, in_maps, core_ids=list(range(N_CORES)), trace=TRACE,
    )
    LAST_RESULTS = res

    out = np.zeros((B, T, D), dtype=np.float64)
    for c in range(N_CORES):
        p = res.results[c]["out_part"].astype(np.float64)
        out[c // 4] += p[0] + p[1]
    out += b_out.astype(np.float64)
    return out.astype(np.float32)



# revision 5
# speedup vs baseline: 1.1508x; 1.1508x over previous
"""Multi-head self-attention with RoPE on 8 Trainium2 NeuronCores.

Full inputs in, full output out. Sharding: batch (2) x head-groups (4 heads
per core). Each core computes qkv projections for its heads, RoPE, full
softmax(QK^T)V, and a combined (both head-pairs) partial output projection;
host sums the 4 partials per batch element and adds b_out.

All matmul operands are bf16 (fp32 PSUM accumulation); the emission order
interleaves the v projection and pair-1 q/k projections into pair-0's
ACT-bound attention stream so the PE stays busy.

Problem shape: B=2, T=2048, D=1024, H=16, HD=64 (hardcoded).
"""

import numpy as np
from contextlib import ExitStack

import ml_dtypes
import concourse.bass as bass
import concourse.mybir as mybir
import concourse.tile as tile
from concourse import bass_utils

B, T, D, H = 2, 2048, 1024, 16
HD = 64          # head dim
HL = 4           # heads per core
N_CORES = 8
ROPE_BASE = 10000.0

F32 = mybir.dt.float32
F32R = mybir.dt.float32r
BF16 = mybir.dt.bfloat16
BFNP = ml_dtypes.bfloat16

Exp = mybir.ActivationFunctionType.Exp

NT = T // 128     # 16 token tiles
NK = D // 128     # 8 contraction chunks
TH2 = 1024        # query-half width
SC = HD ** -0.5

# results of the last run (for test harness introspection)
LAST_RESULTS = None
TRACE = False


def _split_excess_waits(nc, cap=1):
    """walrus in this env rejects >1 sync-wait per instruction; split extras
    onto single-wait NoOps on the same engine queue."""
    n = 0
    for f in nc.m.functions:
        for bb in f.blocks:
            insts = bb.instructions
            if not any(
                i.sync_info is not None and len(i.sync_info.on_wait) > cap
                for i in insts
            ):
                continue
            out = []
            for inst in insts:
                si = inst.sync_info
                waits = list(si.on_wait) if si is not None else []
                if len(waits) > cap:
                    extra, keep = waits[:-cap], waits[-cap:]
                    for k, w in enumerate(extra):
                        nop = mybir.InstNoOp(
                            name=f"{inst.name}-ws{k}",
                            engine=inst.engine,
                            sync_info=mybir.SyncInfo(on_wait=[w], on_update=[]),
                            bass_nofuse=True,
                        )
                        nc.register_instruction(nop)
                        out.append(nop)
                        n += 1
                    inst.sync_info = mybir.SyncInfo(
                        on_wait=keep, on_update=list(si.on_update)
                    )
                out.append(inst)
            bb.instructions = out
    return n


def _build_bass(with_qkv_bias, with_v_bias):
    nc = bass.Bass("TRN2", target_bir_lowering=False, debug=False, num_devices=1)

    # ---- DRAM I/O ----
    d_xT = nc.dram_tensor("xT", [D, T], BF16, kind="ExternalInput").ap()
    d_wqk = nc.dram_tensor("wqk", [D, 4 * 128], BF16, kind="ExternalInput").ap()
    d_wv = nc.dram_tensor("wv", [D, HL * (HD + 1)], BF16, kind="ExternalInput").ap()
    d_bqk = nc.dram_tensor("bqk", [1, 4 * 128], BF16, kind="ExternalInput").ap()
    d_bv = nc.dram_tensor("bv", [1, HL * (HD + 1)], BF16, kind="ExternalInput").ap()
    d_ones = nc.dram_tensor("ones", [1, 512], BF16, kind="ExternalInput").ap()
    d_cos = nc.dram_tensor("cos2", [128, T], F32, kind="ExternalInput").ap()
    d_sin = nc.dram_tensor("sin2", [128, T], F32, kind="ExternalInput").ap()
    d_rT = nc.dram_tensor("rT", [128, 128], BF16, kind="ExternalInput").ap()
    d_ind = nc.dram_tensor("ind", [2, 128], F32R, kind="ExternalInput").ap()
    d_amask = nc.dram_tensor("amask", [128, NT], F32, kind="ExternalInput").ap()
    d_wo = nc.dram_tensor("wo", [2 * 128, D], BF16, kind="ExternalInput").ap()
    d_out = nc.dram_tensor("out_part", [T, D], BF16, kind="ExternalOutput").ap()

    with tile.TileContext(nc) as tc, ExitStack() as ctx:
        pool = lambda name, bufs: ctx.enter_context(tc.tile_pool(name=name, bufs=bufs))
        psum = lambda name, bufs: ctx.enter_context(
            tc.tile_pool(name=name, bufs=bufs, space="PSUM")
        )

        p_const = pool("const", 1)
        p_xt = pool("xt", NK)
        p_w = pool("w", NK)
        p_wv = pool("wv", NK)
        p_cs = pool("cs", 1)
        p_tmp = pool("tmp", 2)
        p_qk = pool("qk", 2)
        p_v = pool("v", NT)
        p_e = pool("e", 4)
        p_at = pool("at", 4)
        p_fin = pool("fin", 2)

        ps_s = psum("ps_s", 2)      # [128,1024] f32 -> 4 banks
        ps_pv = psum("ps_pv", 1)    # [65,1024] f32 -> 2 banks
        ps_aux = psum("ps_aux", 2)  # [128,512] f32 -> 2 banks

        # ---- input loads ----
        # x + wqk interleaved on the sync HWDGE ring (needed first);
        # tables/v/out-proj weights on the scalar ring (ACT idle early).
        xt, wqk_sb = [], []
        for k in range(NK):
            tw = p_w.tile([128, 4 * 128], BF16, tag="wqk", name="wqk_t")
            nc.sync.dma_start(tw[:], d_wqk[k * 128:(k + 1) * 128, :])
            wqk_sb.append(tw)
            tk = p_xt.tile([128, T], BF16, tag="xt", name="xt_t")
            nc.sync.dma_start(tk[:], d_xT[k * 128:(k + 1) * 128, :])
            xt.append(tk)

        t_rT = p_const.tile([128, 128], BF16, tag="rT")
        nc.scalar.dma_start(t_rT[:], d_rT[:])
        t_cos = p_cs.tile([128, T], F32, tag="cos")
        t_sin = p_cs.tile([128, T], F32, tag="sin")
        for ih in range(2):
            s = slice(ih * TH2, (ih + 1) * TH2)
            nc.scalar.dma_start(t_cos[:, s], d_cos[:, s])
            nc.scalar.dma_start(t_sin[:, s], d_sin[:, s])
        t_amask = p_const.tile([128, NT], F32, tag="amask")
        nc.scalar.dma_start(t_amask[:], d_amask[:])
        t_ind2 = p_const.tile([2, 128], F32R, tag="ind")
        nc.scalar.dma_start(t_ind2[:], d_ind[:])
        t_ones = p_const.tile([1, 512], BF16, tag="ones")
        nc.scalar.dma_start(t_ones[:], d_ones[:])
        t_bqk = p_const.tile([1, 4 * 128], BF16, tag="bqk")
        nc.scalar.dma_start(t_bqk[:], d_bqk[:])
        t_bv = p_const.tile([1, HL * (HD + 1)], BF16, tag="bv")
        nc.scalar.dma_start(t_bv[:], d_bv[:])
        wv_sb = []
        for k in range(NK):
            tv = p_wv.tile([128, HL * (HD + 1)], BF16, tag="wv", name="wv_t")
            nc.scalar.dma_start(tv[:], d_wv[k * 128:(k + 1) * 128, :])
            wv_sb.append(tv)
        wo_sb = []
        for c2 in range(2):
            wt = p_fin.tile([128, D], BF16, tag="wo", name="wo_t")
            nc.scalar.dma_start(wt[:], d_wo[c2 * 128:(c2 + 1) * 128, :])
            wo_sb.append(wt)

        # ---- persistent q/k tiles; zero-pad k halves once ----
        qc, kA, kB = [], [], []
        for pair in range(2):
            tq = p_qk.tile([128, T], BF16, tag="qc", name="qc_t")
            ta = p_qk.tile([128, T], BF16, tag="kA", name="kA_t")
            tb = p_qk.tile([128, T], BF16, tag="kB", name="kB_t")
            nc.gpsimd.memset(ta[HD:128, :], 0.0)
            nc.gpsimd.memset(tb[0:HD, :], 0.0)
            qc.append(tq)
            kA.append(ta)
            kB.append(tb)

        v_sb = [None] * NT
        at_t = [None] * 4
        an_t = [None] * 4  # (pair, ih) -> 2*pair + ih

        # ---- emission helpers ----
        def emit_proj_quarter(c2, qi, pair, is_k):
            """project feature chunk c2 for token quarter qi, rope, store."""
            sl = slice(qi * 512, (qi + 1) * 512)
            acc = ps_aux.tile([128, 512], F32, tag="aux", name="acc")
            for k in range(NK):
                nc.tensor.matmul(
                    acc[:],
                    wqk_sb[k][:, c2 * 128:(c2 + 1) * 128],
                    xt[k][:, sl],
                    start=(k == 0),
                    stop=(not with_qkv_bias and k == NK - 1),
                    skip_group_check=True,
                )
            if with_qkv_bias:
                nc.tensor.matmul(
                    acc[:],
                    t_bqk[:, c2 * 128:(c2 + 1) * 128],
                    t_ones[:, 0:512],
                    start=False,
                    stop=True,
                    skip_group_check=True,
                )
            # RoPE: roped = raw*cos + R @ (raw*sin)
            u = p_tmp.tile([128, 512], BF16, tag="u", name="u_t")
            nc.vector.tensor_mul(u[:], acc[:], t_sin[:, sl])
            rot = ps_aux.tile([128, 512], F32, tag="aux", name="rot")
            nc.tensor.matmul(rot[:], t_rT[:], u[:], start=True, stop=True,
                             skip_group_check=True)
            c_sb = p_tmp.tile([128, 512], F32, tag="c", name="c_t")
            nc.vector.tensor_mul(c_sb[:], acc[:], t_cos[:, sl])
            if not is_k:
                nc.vector.tensor_add(qc[pair][:, sl], c_sb[:], rot[:])
            else:
                nc.vector.tensor_add(kA[pair][0:HD, sl], c_sb[0:HD, :],
                                     rot[0:HD, :])
                nc.vector.tensor_add(kB[pair][HD:128, sl], c_sb[HD:128, :],
                                     rot[HD:128, :])

        def emit_v(j):
            acc = ps_aux.tile([128, 512], F32, tag="aux", name="vacc")
            av = acc[:, 0:HL * (HD + 1)]
            for k in range(NK):
                nc.tensor.matmul(
                    av,
                    xt[k][:, j * 128:(j + 1) * 128],
                    wv_sb[k][:],
                    start=(k == 0),
                    stop=(not with_v_bias and k == NK - 1),
                    skip_group_check=True,
                )
            vt = p_v.tile([128, HL * (HD + 1)], BF16, tag="v", name="v_t")
            if with_v_bias:
                nc.tensor.matmul(av, t_ones[:, 0:128], t_bv[:],
                                 start=False, stop=True, skip_group_check=True)
                nc.vector.tensor_copy(vt[:], av)
            else:
                nc.vector.tensor_copy(vt[:], av)
                ones_cols = vt[:].rearrange("p (h c) -> p h c", h=HL)[:, :, HD:HD + 1]
                nc.gpsimd.memset(ones_cols, 1.0)
            v_sb[j] = vt

        def emit_norm(pair, ih):
            """per-(pair, query-half) softmax normalization -> an tile (bf16)."""
            hsl = slice(ih * TH2, (ih + 1) * TH2)
            atA, atB = at_t[2 * pair], at_t[2 * pair + 1]
            sums = p_fin.tile([128, 16], F32, tag="sums", name="sums_t")
            for i, at_ in enumerate((atA, atB)):
                nc.sync.dma_start(
                    sums[:, i * 8:(i + 1) * 8],
                    at_[HD:HD + 1, hsl].rearrange("o (p c) -> o p c", p=128),
                )
            rec = p_fin.tile([128, 16], F32, tag="rec", name="rec_t")
            nc.vector.reciprocal(rec[:], sums[:])
            rec2 = p_fin.tile([2, TH2], F32R, tag="rec2", name="rec2_t")
            for i in range(2):
                nc.sync.dma_start(
                    rec2[i:i + 1, :].rearrange("o (p c) -> o p c", p=128),
                    rec[:, i * 8:(i + 1) * 8].bitcast(F32R),
                )
            ar = p_fin.tile([128, TH2], F32, tag="ar", name="ar_t")
            nc.sync.dma_start(ar[0:HD, :], atA[0:HD, hsl])
            nc.sync.dma_start(ar[HD:128, :], atB[0:HD, hsl])
            an = p_fin.tile([128, TH2], BF16, tag="an", bufs=4, name="an_t")
            for n5 in range(2):
                s5 = slice(n5 * 512, (n5 + 1) * 512)
                pb = ps_aux.tile([128, 512], F32, tag="aux", name="pb")
                nc.tensor.matmul(pb[:], t_ind2[:], rec2[:, s5],
                                 start=True, stop=True, skip_group_check=True)
                nc.vector.tensor_mul(an[:, s5], pb[:], ar[:, s5])
            an_t[2 * pair + ih] = an

        def emit_outproj_tile(t, tail=False):
            """output projection for token tile t, both pairs accumulated."""
            ih = t // 8
            off = (t % 8) * 128
            an0, an1 = an_t[0 + ih], an_t[2 + ih]
            osb = p_fin.tile([128, D], BF16, tag="osb", bufs=3, name="osb_t")
            for n5 in range(2):
                s5 = slice(n5 * 512, (n5 + 1) * 512)
                pp = ps_aux.tile([128, 512], F32, tag="aux", name="pp")
                nc.tensor.matmul(pp[:], an0[:, off:off + 128], wo_sb[0][:, s5],
                                 start=True, stop=False, skip_group_check=True)
                nc.tensor.matmul(pp[:], an1[:, off:off + 128], wo_sb[1][:, s5],
                                 start=False, stop=True, skip_group_check=True)
                if tail and n5 == 1:
                    nc.scalar.copy(osb[:, s5], pp[:])
                else:
                    nc.vector.tensor_copy(osb[:, s5], pp[:])
            nc.sync.dma_start(d_out[t * 128:(t + 1) * 128, :], osb[:])

        def emit_att_step(pair, ih, hh, jb):
            kp = (kA, kB)[hh][pair]
            s_ps = ps_s.tile([128, TH2], F32, tag="sT", name="s_ps")
            for n5 in range(2):
                s5 = slice(n5 * 512, (n5 + 1) * 512)
                g5 = slice(ih * TH2 + n5 * 512, ih * TH2 + (n5 + 1) * 512)
                nc.tensor.matmul(
                    s_ps[:, s5], kp[:, jb * 128:(jb + 1) * 128], qc[pair][:, g5],
                    start=True, stop=True, skip_group_check=True,
                )
            e = p_e.tile([128, TH2], BF16, tag="e", name="e_t")
            nc.scalar.activation(e[:], s_ps[:], Exp,
                                 bias=t_amask[:, jb:jb + 1], scale=SC)
            return s_ps, e

        def emit_pv(pair, hh, jb, pv_ps, e):
            h = 2 * pair + hh
            for n5 in range(2):
                s5 = slice(n5 * 512, (n5 + 1) * 512)
                nc.tensor.matmul(
                    pv_ps[:, s5],
                    v_sb[jb][:, h * (HD + 1):(h + 1) * (HD + 1)],
                    e[:, s5],
                    start=(jb == 0), stop=(jb == NT - 1),
                    skip_group_check=True,
                )

        # ---- phase 1: k0, q0 (first half) projections ----
        for qi in range(4):
            emit_proj_quarter(2, qi, 0, True)     # k pair0
        emit_proj_quarter(0, 0, 0, False)          # q pair0 quarters 0,1 (ih0)
        emit_proj_quarter(0, 1, 0, False)
        emit_v(0)

        # pending interleave units for pair0's attention stream
        pend = []
        pend.append(lambda: emit_proj_quarter(0, 2, 0, False))  # q0 ih1
        pend.append(lambda: emit_proj_quarter(0, 3, 0, False))
        for qi in range(4):
            pend.append(lambda qi=qi: emit_proj_quarter(3, qi, 1, True))   # k1
        for qi in range(4):
            pend.append(lambda qi=qi: emit_proj_quarter(1, qi, 1, False))  # q1

        def drain(n=1):
            for _ in range(n):
                if pend:
                    pend.pop(0)()

        # ---- pair 0 attention ----
        for ih in range(2):
            for hh in range(2):
                if ih == 0:
                    at_t[2 * 0 + hh] = p_at.tile([HD + 1, T], F32, tag="aT",
                                                 name="at_t")
                at = at_t[2 * 0 + hh]
                pv_ps = ps_pv.tile([HD + 1, TH2], F32, tag="pv", name="pv_ps")
                for jb in range(NT):
                    s_ps, e = emit_att_step(0, ih, hh, jb)
                    if ih == 0 and hh == 0:
                        if jb < NT - 1:
                            emit_v(jb + 1)
                        if jb < 2:
                            drain(1)  # q0 ih1 quarters early
                    elif (ih, hh) in ((0, 1), (1, 0)):
                        if jb % 2 == 0:
                            drain(1)  # k1/q1 quarters
                    emit_pv(0, hh, jb, pv_ps, e)
                nc.vector.tensor_copy(at[:, ih * TH2:(ih + 1) * TH2], pv_ps[:])
            if ih == 0:
                emit_norm(0, 0)
        emit_norm(0, 1)

        # ---- pair 1 attention ----
        for ih in range(2):
            for hh in range(2):
                if ih == 0:
                    at_t[2 * 1 + hh] = p_at.tile([HD + 1, T], F32, tag="aT",
                                                 name="at_t")
                at = at_t[2 * 1 + hh]
                pv_ps = ps_pv.tile([HD + 1, TH2], F32, tag="pv", name="pv_ps")
                for jb in range(NT):
                    s_ps, e = emit_att_step(1, ih, hh, jb)
                    if ih == 1 and jb % 4 == 0:
                        # outproj tiles 0..7 spread over ih1's 32 steps
                        emit_outproj_tile(hh * 4 + jb // 4)
                    emit_pv(1, hh, jb, pv_ps, e)
                nc.vector.tensor_copy(at[:, ih * TH2:(ih + 1) * TH2], pv_ps[:])
            if ih == 0:
                emit_norm(1, 0)

        # ---- tail: last norm + remaining outproj ----
        emit_norm(1, 1)
        for t in range(8, NT):
            emit_outproj_tile(t, tail=True)

    _split_excess_waits(nc)
    return nc


_NC_CACHE = {}


def _rope_tables():
    inv_freq = (1.0 / (ROPE_BASE ** (np.arange(0, HD, 2, dtype=np.float32) / HD))
                ).astype(np.float32)
    t = np.arange(T, dtype=np.float32)
    freqs = np.einsum("t,f->tf", t, inv_freq).astype(np.float32)  # (T, HD/2)
    emb = np.concatenate([freqs, freqs], axis=-1)                  # (T, HD)
    cosT = np.cos(emb).astype(np.float32).T                        # (HD, T)
    sinT = np.sin(emb).astype(np.float32).T
    cos2 = np.ascontiguousarray(np.tile(cosT, (2, 1)))             # (128, T)
    sin2 = np.ascontiguousarray(np.tile(sinT, (2, 1)))
    return cos2, sin2


def _rot_matrix():
    r = np.zeros((128, 128), dtype=np.float32)
    for p0 in (0, 64):
        for d in range(32):
            r[p0 + d, p0 + 32 + d] = -1.0
            r[p0 + 32 + d, p0 + d] = 1.0
    return np.ascontiguousarray(r.T)


def kernel(x, W_qkv, b_qkv, W_out, b_out, padding_mask):
    global _NC_CACHE, LAST_RESULTS
    x = np.asarray(x, dtype=np.float32)
    W_qkv = np.asarray(W_qkv, dtype=np.float32)
    b_qkv = np.asarray(b_qkv, dtype=np.float32)
    W_out = np.asarray(W_out, dtype=np.float32)
    b_out = np.asarray(b_out, dtype=np.float32)
    padding_mask = np.asarray(padding_mask)

    with_qkv_bias = bool(np.any(b_qkv[:2 * D]))
    with_v_bias = bool(np.any(b_qkv[2 * D:]))
    key = (with_qkv_bias, with_v_bias)
    if key not in _NC_CACHE:
        _NC_CACHE[key] = _build_bass(with_qkv_bias, with_v_bias)
    nc = _NC_CACHE[key]

    cos2, sin2 = _rope_tables()
    rT = _rot_matrix().astype(BFNP)

    ind = np.zeros((2, 128), dtype=np.float32)
    for f in range(128):
        ind[f // 64, f] = 1.0

    ones = np.ones((1, 512), dtype=BFNP)

    in_maps = []
    for c in range(N_CORES):
        b = c // 4
        g = c % 4
        q0 = g * HL * HD
        wq = W_qkv[:, q0:q0 + HL * HD]
        wk = W_qkv[:, D + q0:D + q0 + HL * HD]
        wv_flat = W_qkv[:, 2 * D + q0:2 * D + q0 + HL * HD]
        # interleave v columns with a zero (ones-slot) column per head
        wv_aug = np.zeros((D, HL * (HD + 1)), dtype=np.float32)
        bv_aug = np.zeros((1, HL * (HD + 1)), dtype=np.float32)
        for h in range(HL):
            wv_aug[:, h * (HD + 1):h * (HD + 1) + HD] = wv_flat[:, h * HD:(h + 1) * HD]
            bv_aug[0, h * (HD + 1):h * (HD + 1) + HD] = \
                b_qkv[2 * D + q0 + h * HD:2 * D + q0 + (h + 1) * HD]
            bv_aug[0, h * (HD + 1) + HD] = 1.0
        bqk = np.concatenate(
            [b_qkv[q0:q0 + HL * HD], b_qkv[D + q0:D + q0 + HL * HD]]
        ).reshape(1, -1).astype(np.float32)
        amask = np.where(padding_mask[b], np.float32(-1e30), np.float32(0.0))
        amask = np.ascontiguousarray(amask.reshape(T // 128, 128).T.astype(np.float32))
        in_maps.append({
            "xT": np.ascontiguousarray(x[b].T).astype(BFNP),
            "wqk": np.ascontiguousarray(
                np.concatenate([wq, wk], axis=1)).astype(BFNP),
            "wv": wv_aug.astype(BFNP),
            "bqk": bqk.astype(BFNP),
            "bv": bv_aug.astype(BFNP),
            "ones": ones,
            "cos2": cos2,
            "sin2": sin2,
            "rT": rT,
            "ind": ind,
            "amask": amask,
            "wo": np.ascontiguousarray(W_out[q0:q0 + HL * HD, :]).astype(BFNP),
        })

    res = bass_utils.run_bass_kernel_spmd(
        nc, in_maps, core_ids=list(range(N_CORES)), trace=TRACE,
    )
    LAST_RESULTS = res

    out = np.zeros((B, T, D), dtype=np.float32)
    for c in range(N_CORES):
        out[c // 4] += res.results[c]["out_part"].astype(np.float32)
    out += b_out.astype(np.float32)
    return out.astype(np.float32)


# revision 15
# speedup vs baseline: 1.2346x; 1.0727x over previous
"""Multi-head self-attention with RoPE on 8 Trainium2 NeuronCores.

Full inputs in, full output out. Sharding: batch (2) x head-groups (4 heads
per core). Each core computes qkv projections for its heads, RoPE, full
softmax(QK^T)V, and a combined (both head-pairs) partial output projection;
host sums the 4 partials per batch element and adds b_out.

All matmul operands are bf16 (fp32 PSUM accumulation); the emission order
interleaves the v projection and pair-1 q/k projections into pair-0's
ACT-bound attention stream so the PE stays busy.

Problem shape: B=2, T=2048, D=1024, H=16, HD=64 (hardcoded).
"""

import numpy as np
from contextlib import ExitStack

import ml_dtypes
import concourse.bass as bass
import concourse.mybir as mybir
import concourse.tile as tile
from concourse import bass_utils

B, T, D, H = 2, 2048, 1024, 16
HD = 64          # head dim
HL = 4           # heads per core
N_CORES = 8
ROPE_BASE = 10000.0

F32 = mybir.dt.float32
F32R = mybir.dt.float32r
BF16 = mybir.dt.bfloat16
BFNP = ml_dtypes.bfloat16

Exp = mybir.ActivationFunctionType.Exp

NT = T // 128     # 16 token tiles
NK = D // 128     # 8 contraction chunks
TH2 = 1024        # query-half width
SC = HD ** -0.5

# results of the last run (for test harness introspection)
LAST_RESULTS = None
TRACE = False


def _split_excess_waits(nc, cap=1):
    """walrus in this env rejects >1 sync-wait per instruction; split extras
    onto single-wait NoOps on the same engine queue."""
    n = 0
    for f in nc.m.functions:
        for bb in f.blocks:
            insts = bb.instructions
            if not any(
                i.sync_info is not None and len(i.sync_info.on_wait) > cap
                for i in insts
            ):
                continue
            out = []
            for inst in insts:
                si = inst.sync_info
                waits = list(si.on_wait) if si is not None else []
                if len(waits) > cap:
                    extra, keep = waits[:-cap], waits[-cap:]
                    for k, w in enumerate(extra):
                        nop = mybir.InstNoOp(
                            name=f"{inst.name}-ws{k}",
                            engine=inst.engine,
                            sync_info=mybir.SyncInfo(on_wait=[w], on_update=[]),
                            bass_nofuse=True,
                        )
                        nc.register_instruction(nop)
                        out.append(nop)
                        n += 1
                    inst.sync_info = mybir.SyncInfo(
                        on_wait=keep, on_update=list(si.on_update)
                    )
                out.append(inst)
            bb.instructions = out
    return n


def _build_bass(with_qkv_bias, with_v_bias):
    nc = bass.Bass("TRN2", target_bir_lowering=False, debug=False, num_devices=1)

    # ---- DRAM I/O ----
    d_xT = nc.dram_tensor("xT", [D, T], BF16, kind="ExternalInput").ap()
    d_wqk = nc.dram_tensor("wqk", [D, 4 * 128], BF16, kind="ExternalInput").ap()
    d_wv = nc.dram_tensor("wv", [D, HL * (HD + 1)], BF16, kind="ExternalInput").ap()
    d_bqk = nc.dram_tensor("bqk", [1, 4 * 128], BF16, kind="ExternalInput").ap()
    d_bv = nc.dram_tensor("bv", [1, HL * (HD + 1)], BF16, kind="ExternalInput").ap()
    d_ones = nc.dram_tensor("ones", [1, 512], BF16, kind="ExternalInput").ap()
    d_cos = nc.dram_tensor("cos2", [HD, T], F32, kind="ExternalInput").ap()
    d_sin = nc.dram_tensor("sin2", [HD, T], F32, kind="ExternalInput").ap()
    d_rT = nc.dram_tensor("rT", [128, 128], BF16, kind="ExternalInput").ap()
    d_ind = nc.dram_tensor("ind", [2, 128], F32R, kind="ExternalInput").ap()
    d_amask = nc.dram_tensor("amask", [128, NT], F32, kind="ExternalInput").ap()
    d_wo = nc.dram_tensor("wo", [2 * 128, D], BF16, kind="ExternalInput").ap()
    d_out = nc.dram_tensor("out_part", [T, D], BF16, kind="ExternalOutput").ap()

    with tile.TileContext(nc) as tc, ExitStack() as ctx:
        pool = lambda name, bufs: ctx.enter_context(tc.tile_pool(name=name, bufs=bufs))
        psum = lambda name, bufs: ctx.enter_context(
            tc.tile_pool(name=name, bufs=bufs, space="PSUM")
        )

        p_const = pool("const", 1)
        p_xt = pool("xt", NK)
        p_w = pool("w", NK)
        p_wv = pool("wv", NK)
        p_cs = pool("cs", 1)
        p_tmp = pool("tmp", 2)
        p_qk = pool("qk", 2)
        p_v = pool("v", NT)
        p_e = pool("e", 4)
        p_at = pool("at", 4)
        p_fin = pool("fin", 2)

        ps_s = psum("ps_s", 2)      # [128,1024] f32 -> 4 banks
        ps_pv = psum("ps_pv", 1)    # [65,1024] f32 -> 2 banks
        ps_aux = psum("ps_aux", 2)  # [128,512] f32 -> 2 banks

        # ---- input loads ----
        # x arrives in column-batches of 512 tokens via big rearranged
        # descriptors: batch qi unlocks the full contraction for token
        # quarter qi across every projection, so the PE starts ~8us in.
        # wqk rides the sync ring first; tables on the scalar ring.
        xt_all = p_xt.tile([128, NK * T], BF16, tag="xt", bufs=1, name="xt_all")
        wqk_all = p_w.tile([128, NK * 512], BF16, tag="wqk", bufs=1,
                           name="wqk_all")
        nc.sync.dma_start(
            wqk_all[:].rearrange("p (c w) -> p c w", c=NK),
            d_wqk[:].rearrange("(c p) w -> p c w", p=128),
        )
        xt3 = xt_all[:].rearrange("p (c w) -> p c w", c=NK)
        xsrc = d_xT[:].rearrange("(c p) w -> p c w", p=128)
        for qi in range(4):
            for half in range(2):
                cs = slice(half * 4, (half + 1) * 4)
                ws = slice(qi * 512, (qi + 1) * 512)
                nc.sync.dma_start(xt3[:, cs, ws], xsrc[:, cs, ws])

        def xt(k):
            return xt_all[:, k * T:(k + 1) * T]

        def wqk_sb(k):
            return wqk_all[:, k * 512:(k + 1) * 512]

        t_rT = p_const.tile([128, 128], BF16, tag="rT")
        nc.scalar.dma_start(t_rT[:], d_rT[:])
        t_cos = p_cs.tile([128, T], F32, tag="cos")
        t_sin = p_cs.tile([128, T], F32, tag="sin")
        nc.scalar.dma_start(t_cos[0:HD, :], d_cos[:])
        nc.scalar.dma_start(t_sin[0:HD, :], d_sin[:])
        nc.scalar.dma_start(t_cos[HD:128, :], t_cos[0:HD, :])
        nc.scalar.dma_start(t_sin[HD:128, :], t_sin[0:HD, :])
        t_amask = p_const.tile([128, NT], F32, tag="amask")
        nc.scalar.dma_start(t_amask[:], d_amask[:])
        t_ind2 = p_const.tile([2, 128], F32R, tag="ind")
        nc.scalar.dma_start(t_ind2[:], d_ind[:])
        t_ones = p_const.tile([1, 512], BF16, tag="ones")
        nc.scalar.dma_start(t_ones[:], d_ones[:])
        t_bqk = p_const.tile([1, 4 * 128], BF16, tag="bqk")
        nc.scalar.dma_start(t_bqk[:], d_bqk[:])
        t_bv = p_const.tile([1, HL * (HD + 1)], BF16, tag="bv")
        nc.scalar.dma_start(t_bv[:], d_bv[:])
        wv_all = p_wv.tile([128, NK * 260], BF16, tag="wv", bufs=1,
                           name="wv_all")
        nc.scalar.dma_start(
            wv_all[:].rearrange("p (c w) -> p c w", c=NK),
            d_wv[:].rearrange("(c p) w -> p c w", p=128),
        )

        def wv_sb(k):
            return wv_all[:, k * 260:(k + 1) * 260]

        # out-proj weights: not needed until late; sync ring after x
        wo_sb = []
        for c2 in range(2):
            wt = p_fin.tile([128, D], BF16, tag="wo", name="wo_t")
            nc.sync.dma_start(wt[:], d_wo[c2 * 128:(c2 + 1) * 128, :])
            wo_sb.append(wt)

        # ---- persistent q/k tiles; zero-pad k halves once ----
        qc, kA, kB = [], [], []
        for pair in range(2):
            tq = p_qk.tile([128, T], BF16, tag="qc", name="qc_t")
            ta = p_qk.tile([128, T], BF16, tag="kA", name="kA_t")
            tb = p_qk.tile([128, T], BF16, tag="kB", name="kB_t")
            nc.gpsimd.memset(ta[HD:128, :], 0.0)
            nc.gpsimd.memset(tb[0:HD, :], 0.0)
            qc.append(tq)
            kA.append(ta)
            kB.append(tb)

        v_sb = [None] * NT
        at_t = [None] * 4
        an_t = [None] * 4  # (pair, ih) -> 2*pair + ih

        # ---- emission helpers ----
        def emit_proj_mms(acc, c2, sl):
            for k in range(NK):
                nc.tensor.matmul(
                    acc,
                    wqk_sb(k)[:, c2 * 128:(c2 + 1) * 128],
                    xt(k)[:, sl],
                    start=(k == 0),
                    stop=(not with_qkv_bias and k == NK - 1),
                    skip_group_check=True,
                )
            if with_qkv_bias:
                nc.tensor.matmul(
                    acc,
                    t_bqk[:, c2 * 128:(c2 + 1) * 128],
                    t_ones[:, 0:512],
                    start=False,
                    stop=True,
                    skip_group_check=True,
                )

        def emit_rope(acc, qi, pair, is_k, rot_ring="aux"):
            """RoPE: roped = raw*cos + R @ (raw*sin); store q/k bf16."""
            sl = slice(qi * 512, (qi + 1) * 512)
            u = p_tmp.tile([128, 512], BF16, tag="u", name="u_t")
            nc.vector.tensor_mul(u[:], acc, t_sin[:, sl])
            if rot_ring == "pv":
                rot = ps_pv.tile([128, 512], F32, tag="pv", name="rot")
            else:
                rot = ps_aux.tile([128, 512], F32, tag="aux", name="rot")
            nc.tensor.matmul(rot[:], t_rT[:], u[:], start=True, stop=True,
                             skip_group_check=True)
            c_sb = p_tmp.tile([128, 512], F32, tag="c", name="c_t")
            nc.vector.tensor_mul(c_sb[:], acc, t_cos[:, sl])
            if not is_k:
                nc.vector.tensor_add(qc[pair][:, sl], c_sb[:], rot[:])
            else:
                nc.vector.tensor_add(kA[pair][0:HD, sl], c_sb[0:HD, :],
                                     rot[0:HD, :])
                nc.vector.tensor_add(kB[pair][HD:128, sl], c_sb[HD:128, :],
                                     rot[HD:128, :])

        def emit_proj_quarter(c2, qi, pair, is_k):
            """interleaved-unit variant: acc+rot from the aux ring."""
            acc = ps_aux.tile([128, 512], F32, tag="aux", name="acc")
            emit_proj_mms(acc[:], c2, slice(qi * 512, (qi + 1) * 512))
            emit_rope(acc[:], qi, pair, is_k, rot_ring="aux")

        def emit_v(j):
            acc = ps_aux.tile([128, 512], F32, tag="aux", name="vacc")
            av = acc[:, 0:HL * (HD + 1)]
            for k in range(NK):
                nc.tensor.matmul(
                    av,
                    xt(k)[:, j * 128:(j + 1) * 128],
                    wv_sb(k)[:],
                    start=(k == 0),
                    stop=(not with_v_bias and k == NK - 1),
                    skip_group_check=True,
                )
            vt = p_v.tile([128, HL * (HD + 1)], BF16, tag="v", name="v_t")
            if with_v_bias:
                nc.tensor.matmul(av, t_ones[:, 0:128], t_bv[:],
                                 start=False, stop=True, skip_group_check=True)
                nc.vector.tensor_copy(vt[:], av)
            else:
                nc.vector.tensor_copy(vt[:], av)
                ones_cols = vt[:].rearrange("p (h c) -> p h c", h=HL)[:, :, HD:HD + 1]
                nc.gpsimd.memset(ones_cols, 1.0)
            v_sb[j] = vt

        norm_state = {}

        def emit_norm_head(pair, ih, hh):
            """emit the per-head part of softmax normalization right after
            head hh's attention half completes, so its DMA latency hides
            behind the other head's attention stream."""
            hsl = slice(ih * TH2, (ih + 1) * TH2)
            at_ = at_t[2 * pair + hh]
            if hh == 0:
                sums = p_fin.tile([128, 16], F32, tag="sums", name="sums_t")
                rec = p_fin.tile([128, 16], F32, tag="rec", name="rec_t")
                rec2 = p_fin.tile([2, TH2], F32R, tag="rec2", name="rec2_t")
                ar = p_fin.tile([128, TH2], F32, tag="ar", name="ar_t")
                norm_state[(pair, ih)] = (sums, rec, rec2, ar)
            else:
                sums, rec, rec2, ar = norm_state[(pair, ih)]
            nc.sync.dma_start(
                sums[:, hh * 8:(hh + 1) * 8],
                at_[HD:HD + 1, hsl].rearrange("o (p c) -> o p c", p=128),
            )
            nc.vector.reciprocal(rec[:, hh * 8:(hh + 1) * 8],
                                 sums[:, hh * 8:(hh + 1) * 8])
            nc.sync.dma_start(
                rec2[hh:hh + 1, :].rearrange("o (p c) -> o p c", p=128),
                rec[:, hh * 8:(hh + 1) * 8].bitcast(F32R),
            )
            nc.sync.dma_start(ar[hh * HD:(hh + 1) * HD, :], at_[0:HD, hsl])

        def emit_norm_fin(pair, ih):
            _, _, rec2, ar = norm_state.pop((pair, ih))
            an = p_fin.tile([128, TH2], BF16, tag="an", bufs=4, name="an_t")
            for n5 in range(2):
                s5 = slice(n5 * 512, (n5 + 1) * 512)
                pb = ps_aux.tile([128, 512], F32, tag="aux", name="pb")
                nc.tensor.matmul(pb[:], t_ind2[:], rec2[:, s5],
                                 start=True, stop=True, skip_group_check=True)
                nc.vector.tensor_mul(an[:, s5], pb[:], ar[:, s5])
            an_t[2 * pair + ih] = an

        def emit_outproj_tile(t, tail=False):
            """output projection for token tile t, both pairs accumulated."""
            ih = t // 8
            off = (t % 8) * 128
            an0, an1 = an_t[0 + ih], an_t[2 + ih]
            osb = p_fin.tile([128, D], BF16, tag="osb", bufs=3, name="osb_t")
            for n5 in range(2):
                s5 = slice(n5 * 512, (n5 + 1) * 512)
                pp = ps_aux.tile([128, 512], F32, tag="aux", name="pp")
                nc.tensor.matmul(pp[:], an0[:, off:off + 128], wo_sb[0][:, s5],
                                 start=True, stop=False, skip_group_check=True)
                nc.tensor.matmul(pp[:], an1[:, off:off + 128], wo_sb[1][:, s5],
                                 start=False, stop=True, skip_group_check=True)
                if tail and n5 == 1:
                    nc.scalar.copy(osb[:, s5], pp[:])
                else:
                    nc.vector.tensor_copy(osb[:, s5], pp[:])
            nc.sync.dma_start(d_out[t * 128:(t + 1) * 128, :], osb[:])

        def emit_att_step(pair, ih, hh, jb):
            kp = (kA, kB)[hh][pair]
            s_ps = ps_s.tile([128, TH2], F32, tag="sT", name="s_ps")
            for n5 in range(2):
                s5 = slice(n5 * 512, (n5 + 1) * 512)
                g5 = slice(ih * TH2 + n5 * 512, ih * TH2 + (n5 + 1) * 512)
                nc.tensor.matmul(
                    s_ps[:, s5], kp[:, jb * 128:(jb + 1) * 128], qc[pair][:, g5],
                    start=True, stop=True, skip_group_check=True,
                )
            e = p_e.tile([128, TH2], BF16, tag="e", name="e_t")
            nc.scalar.activation(e[:], s_ps[:], Exp,
                                 bias=t_amask[:, jb:jb + 1], scale=SC)
            return s_ps, e

        def emit_pv(pair, hh, jb, pv_ps, e):
            h = 2 * pair + hh
            for n5 in range(2):
                s5 = slice(n5 * 512, (n5 + 1) * 512)
                nc.tensor.matmul(
                    pv_ps[:, s5],
                    v_sb[jb][:, h * (HD + 1):(h + 1) * (HD + 1)],
                    e[:, s5],
                    start=(jb == 0), stop=(jb == NT - 1),
                    skip_group_check=True,
                )

        # ---- prologue: k0 + q0 for token halves 0 (x batches 0,1) ----
        # acc slots borrowed from the (still idle) scores ring so four
        # accumulations pipeline; rot slots borrowed from the pv ring.
        for qi in range(2):
            big = ps_s.tile([128, TH2], F32, tag="sT", name="acc_big")
            aK = big[:, 0:512]
            aQ = big[:, 512:1024]
            emit_proj_mms(aK, 2, slice(qi * 512, (qi + 1) * 512))
            emit_proj_mms(aQ, 0, slice(qi * 512, (qi + 1) * 512))
            emit_rope(aK, qi, 0, True, rot_ring="pv")
            emit_rope(aQ, qi, 0, False, rot_ring="pv")
        emit_v(0)

        # pending interleave units for pair0's attention stream
        pend = []
        pend.append(lambda: emit_proj_quarter(2, 2, 0, True))   # k0 q2 (jb>=8)
        pend.append(lambda: emit_proj_quarter(2, 3, 0, True))   # k0 q3 (jb>=12)
        pend.append(lambda: emit_proj_quarter(0, 2, 0, False))  # q0 ih1
        pend.append(lambda: emit_proj_quarter(0, 3, 0, False))
        for qi in range(4):
            pend.append(lambda qi=qi: emit_proj_quarter(3, qi, 1, True))   # k1
        for qi in range(4):
            pend.append(lambda qi=qi: emit_proj_quarter(1, qi, 1, False))  # q1

        def drain(n=1):
            for _ in range(n):
                if pend:
                    pend.pop(0)()

        # ---- pair 0 attention ----
        for ih in range(2):
            for hh in range(2):
                if ih == 0:
                    at_t[2 * 0 + hh] = p_at.tile([HD + 1, T], F32, tag="aT",
                                                 name="at_t")
                at = at_t[2 * 0 + hh]
                pv_ps = ps_pv.tile([HD + 1, TH2], F32, tag="pv", name="pv_ps")
                for jb in range(NT):
                    s_ps, e = emit_att_step(0, ih, hh, jb)
                    if ih == 0 and hh == 0:
                        if jb < NT - 1:
                            emit_v(jb + 1)
                        if jb in (1, 5, 9, 13):
                            drain(1)  # k0 q2/q3 ahead of jb 8/12, then q0 ih1
                    elif jb % 4 == 0:
                        drain(1)  # k1/q1 quarters, evenly spread
                    emit_pv(0, hh, jb, pv_ps, e)
                nc.vector.tensor_copy(at[:, ih * TH2:(ih + 1) * TH2], pv_ps[:])
                emit_norm_head(0, ih, hh)
            emit_norm_fin(0, ih)

        # ---- pair 1 attention ----
        for ih in range(2):
            for hh in range(2):
                if ih == 0:
                    at_t[2 * 1 + hh] = p_at.tile([HD + 1, T], F32, tag="aT",
                                                 name="at_t")
                at = at_t[2 * 1 + hh]
                pv_ps = ps_pv.tile([HD + 1, TH2], F32, tag="pv", name="pv_ps")
                for jb in range(NT):
                    s_ps, e = emit_att_step(1, ih, hh, jb)
                    if ih == 0 and jb % 8 == 4:
                        drain(1)  # any leftover proj units
                    if ih == 1 and jb % 4 == 0:
                        # outproj tiles 0..7 spread over ih1's 32 steps
                        emit_outproj_tile(hh * 4 + jb // 4)
                    emit_pv(1, hh, jb, pv_ps, e)
                nc.vector.tensor_copy(at[:, ih * TH2:(ih + 1) * TH2], pv_ps[:])
                emit_norm_head(1, ih, hh)
            emit_norm_fin(1, ih)

        # ---- tail: remaining outproj ----
        for t in range(8, NT):
            emit_outproj_tile(t, tail=True)

    _split_excess_waits(nc)
    return nc


_NC_CACHE = {}


def _rope_tables():
    inv_freq = (1.0 / (ROPE_BASE ** (np.arange(0, HD, 2, dtype=np.float32) / HD))
                ).astype(np.float32)
    t = np.arange(T, dtype=np.float32)
    freqs = np.einsum("t,f->tf", t, inv_freq).astype(np.float32)  # (T, HD/2)
    emb = np.concatenate([freqs, freqs], axis=-1)                  # (T, HD)
    cosT = np.ascontiguousarray(np.cos(emb).astype(np.float32).T)  # (HD, T)
    sinT = np.ascontiguousarray(np.sin(emb).astype(np.float32).T)
    return cosT, sinT


def _rot_matrix():
    r = np.zeros((128, 128), dtype=np.float32)
    for p0 in (0, 64):
        for d in range(32):
            r[p0 + d, p0 + 32 + d] = -1.0
            r[p0 + 32 + d, p0 + d] = 1.0
    return np.ascontiguousarray(r.T)


def kernel(x, W_qkv, b_qkv, W_out, b_out, padding_mask):
    global _NC_CACHE, LAST_RESULTS
    x = np.asarray(x, dtype=np.float32)
    W_qkv = np.asarray(W_qkv, dtype=np.float32)
    b_qkv = np.asarray(b_qkv, dtype=np.float32)
    W_out = np.asarray(W_out, dtype=np.float32)
    b_out = np.asarray(b_out, dtype=np.float32)
    padding_mask = np.asarray(padding_mask)

    with_qkv_bias = bool(np.any(b_qkv[:2 * D]))
    with_v_bias = bool(np.any(b_qkv[2 * D:]))
    key = (with_qkv_bias, with_v_bias)
    if key not in _NC_CACHE:
        _NC_CACHE[key] = _build_bass(with_qkv_bias, with_v_bias)
    nc = _NC_CACHE[key]

    cos2, sin2 = _rope_tables()
    rT = _rot_matrix().astype(BFNP)

    ind = np.zeros((2, 128), dtype=np.float32)
    for f in range(128):
        ind[f // 64, f] = 1.0

    ones = np.ones((1, 512), dtype=BFNP)

    in_maps = []
    for c in range(N_CORES):
        b = c // 4
        g = c % 4
        q0 = g * HL * HD
        wq = W_qkv[:, q0:q0 + HL * HD]
        wk = W_qkv[:, D + q0:D + q0 + HL * HD]
        wv_flat = W_qkv[:, 2 * D + q0:2 * D + q0 + HL * HD]
        # interleave v columns with a zero (ones-slot) column per head
        wv_aug = np.zeros((D, HL * (HD + 1)), dtype=np.float32)
        bv_aug = np.zeros((1, HL * (HD + 1)), dtype=np.float32)
        for h in range(HL):
            wv_aug[:, h * (HD + 1):h * (HD + 1) + HD] = wv_flat[:, h * HD:(h + 1) * HD]
            bv_aug[0, h * (HD + 1):h * (HD + 1) + HD] = \
                b_qkv[2 * D + q0 + h * HD:2 * D + q0 + (h + 1) * HD]
            bv_aug[0, h * (HD + 1) + HD] = 1.0
        bqk = np.concatenate(
            [b_qkv[q0:q0 + HL * HD], b_qkv[D + q0:D + q0 + HL * HD]]
        ).reshape(1, -1).astype(np.float32)
        amask = np.where(padding_mask[b], np.float32(-1e30), np.float32(0.0))
        amask = np.ascontiguousarray(amask.reshape(T // 128, 128).T.astype(np.float32))
        in_maps.append({
            "xT": np.ascontiguousarray(x[b].T).astype(BFNP),
            "wqk": np.ascontiguousarray(
                np.concatenate([wq, wk], axis=1)).astype(BFNP),
            "wv": wv_aug.astype(BFNP),
            "bqk": bqk.astype(BFNP),
            "bv": bv_aug.astype(BFNP),
            "ones": ones,
            "cos2": cos2,
            "sin2": sin2,
            "rT": rT,
            "ind": ind,
            "amask": amask,
            "wo": np.ascontiguousarray(W_out[q0:q0 + HL * HD, :]).astype(BFNP),
        })

    res = bass_utils.run_bass_kernel_spmd(
        nc, in_maps, core_ids=list(range(N_CORES)), trace=TRACE,
    )
    LAST_RESULTS = res

    out = np.zeros((B, T, D), dtype=np.float32)
    for c in range(N_CORES):
        out[c // 4] += res.results[c]["out_part"].astype(np.float32)
    out += b_out.astype(np.float32)
    return out.astype(np.float32)
